# revision 5
# baseline (speedup 1.0000x reference)
"""AttentionBlock (GroupNorm + 1x1 QKV + MHA + proj + residual) on 8 trn2 cores.

Sharding: core c -> (batch b = c//2, t-half = c%2). Each core computes all 4
heads for its 2048 query positions; k/v are computed over the full T=4096 from
the core's batch. No cross-core communication needed.

Layout trick: attention scores are computed TRANSPOSED, S_T[s, t] (s on
partitions), so the AV matmul needs no transposes: a[ch, t] = vT[s, ch].T @
P_T[s, t]. The softmax denominator comes from a ones-column appended to vT.
GroupNorm is folded into the QKV weights (per-channel scale/shift).
"""

import math

import numpy as np

import concourse.bass as bass
import concourse.tile as tile
from concourse import bacc, mybir
from concourse import bass_utils

F32 = mybir.dt.float32
F32R = mybir.dt.float32r
BF16 = mybir.dt.bfloat16

B, C, HH, WW = 4, 256, 64, 64
T = HH * WW            # 4096
NH = 4                 # heads per batch
CH = C // NH           # 64 channels per head
G = 32                 # groupnorm groups
CPG = C // G           # 8 channels per group
EPS = 1e-5
NCORES = 8
THALF = T // 2         # 2048  t-columns per core
SCALE = 1.0 / math.sqrt(math.sqrt(CH))

USE_F32R = True        # f32r (tf32-like) for the big matmuls
P_DT = BF16            # dtype of exp'd attention weights + vT (bf16 or f32)

_CACHE = {}
LAST_RESULTS = None


def _r(ap):
    return ap


def _build_program():
    nc = bacc.Bacc("TRN2", target_bir_lowering=False, debug=False)

    d_xb = nc.dram_tensor("xb", [C, T], F32R, kind="ExternalInput").ap()
    d_xq = nc.dram_tensor("xq", [C, THALF], F32R, kind="ExternalInput").ap()
    d_wT = nc.dram_tensor("wT", [C, 3 * C], F32, kind="ExternalInput").ap()
    d_qkvb = nc.dram_tensor("qkvb", [3 * C, 1], F32, kind="ExternalInput").ap()
    d_vbrow = nc.dram_tensor("vbrow", [1, C], F32, kind="ExternalInput").ap()
    d_pjT = nc.dram_tensor("pjT", [C, C], F32R, kind="ExternalInput").ap()
    d_pjb = nc.dram_tensor("pjb", [C, 1], F32, kind="ExternalInput").ap()
    d_gnw = nc.dram_tensor("gnw", [C, 1], F32, kind="ExternalInput").ap()
    d_gnb = nc.dram_tensor("gnb", [C, 1], F32, kind="ExternalInput").ap()
    d_gsel = nc.dram_tensor("gsel", [128, 2 * G], F32, kind="ExternalInput").ap()
    d_bsel = nc.dram_tensor("bsel", [G, C], F32, kind="ExternalInput").ap()
    d_out = nc.dram_tensor("out", [C, THALF], F32, kind="ExternalOutput").ap()

    with tile.TileContext(nc) as tc:
        with tc.tile_pool(name="data", bufs=1) as data, \
             tc.tile_pool(name="small", bufs=1) as small, \
             tc.tile_pool(name="work", bufs=3) as work:

            # ---- persistent SBUF tensors ----
            x = [data.tile([128, T], F32R, tag=f"x{i}", name=f"x{i}") for i in range(2)]
            xq = [data.tile([128, THALF], F32R, tag=f"xq{i}", name=f"xq{i}") for i in range(2)]
            wt = [data.tile([128, 3 * C], F32, tag=f"wt{i}", name=f"wt{i}") for i in range(2)]
            wtf = [data.tile([128, 3 * C], F32R, tag=f"wtf{i}", name=f"wtf{i}") for i in range(2)]
            pjt = [data.tile([128, C], F32R, tag=f"pjt{i}", name=f"pjt{i}") for i in range(2)]
            q_sb = [data.tile([128, THALF], F32R, tag=f"q{i}", name=f"q{i}") for i in range(2)]
            k_sb = [data.tile([128, T], F32R, tag=f"k{i}", name=f"k{i}") for i in range(2)]
            # vT: per s-chunk, per head: 64 v-columns + 1 ones-column (+3 pad)
            vt = data.tile([128, T // 128, NH, CH + 4], P_DT, tag="vt", name="vt")
            a_sb = [data.tile([128, THALF], F32R, tag=f"a{i}", name=f"a{i}") for i in range(2)]
            vbias_bc = data.tile([128, C], F32, tag="vbias_bc", name="vbias_bc")

            gnw = [small.tile([128, 1], F32, tag=f"gnw{i}", name=f"gnw{i}") for i in range(2)]
            gnb = [small.tile([128, 1], F32, tag=f"gnb{i}", name=f"gnb{i}") for i in range(2)]
            pjb = [small.tile([128, 1], F32, tag=f"pjb{i}", name=f"pjb{i}") for i in range(2)]
            qkb = [small.tile([128, 1], F32, tag=f"qkb{o}", name=f"qkb{o}") for o in range(4)]
            gsel = small.tile([128, 2 * G], F32, tag="gsel", name="gsel")
            bsel = small.tile([G, C], F32, tag="bsel", name="bsel")
            vbrow = small.tile([1, C], F32, tag="vbrow", name="vbrow")

            for i in range(2):
                cs = slice(128 * i, 128 * (i + 1))
                nc.sync.dma_start(x[i][:], d_xb[cs, :])
                nc.sync.dma_start(xq[i][:], d_xq[cs, :])
                nc.sync.dma_start(wt[i][:], d_wT[cs, :])
                nc.sync.dma_start(pjt[i][:], d_pjT[cs, :])
                nc.sync.dma_start(gnw[i][:], d_gnw[cs, :])
                nc.sync.dma_start(gnb[i][:], d_gnb[cs, :])
                nc.sync.dma_start(pjb[i][:], d_pjb[cs, :])
            for o in range(4):
                nc.sync.dma_start(qkb[o][:], d_qkvb[128 * o:128 * (o + 1), :])
            nc.sync.dma_start(gsel[:], d_gsel[:, :])
            nc.sync.dma_start(bsel[:], d_bsel[:, :])
            nc.sync.dma_start(vbrow[:], d_vbrow[:, :])

            # ================= GroupNorm stats -> per-channel A, B ==========
            with tc.tile_pool(name="psS", bufs=1, space="PSUM") as psS, \
                 tc.tile_pool(name="psM", bufs=2, space="PSUM") as psM:

                A = [small.tile([128, 1], F32, tag=f"A{i}", name=f"A{i}") for i in range(2)]
                Bs = [small.tile([128, 1], F32, tag=f"B{i}", name=f"B{i}") for i in range(2)]
                pcs = [small.tile([128, 2], F32, tag=f"pcs{i}", name=f"pcs{i}") for i in range(2)]
                for i in range(2):
                    stats = work.tile([128, 8, 6], F32, tag="bnstats", name="bnstats")
                    for j in range(8):
                        nc.vector.bn_stats(stats[:, j, :], x[i][:, 512 * j:512 * (j + 1)].bitcast(F32))
                    mv = work.tile([128, 2], F32, tag="bnmv", name="bnmv")
                    nc.vector.bn_aggr(mv[:], stats[:])
                    # pcs = (mean, E[x^2]) per channel
                    nc.vector.tensor_copy(pcs[i][:, 0:1], mv[:, 0:1])
                    nc.vector.tensor_tensor(pcs[i][:, 1:2], mv[:, 0:1], mv[:, 0:1],
                                            mybir.AluOpType.mult)
                    nc.vector.tensor_tensor(pcs[i][:, 1:2], pcs[i][:, 1:2], mv[:, 1:2],
                                            mybir.AluOpType.add)

                # group stats [G, 2] = (mean_g, E[x^2]_g)
                grp_ps = psS.tile([G, 2], F32, tag="s", name="s")
                nc.tensor.matmul(grp_ps[:], gsel[:, 0:G], pcs[0][:], start=True, stop=False)
                nc.tensor.matmul(grp_ps[:], gsel[:, G:2 * G], pcs[1][:], start=False, stop=True)

                grp_sb = small.tile([G, 2], F32, tag="grp_sb", name="grp_sb")
                nc.vector.tensor_copy(grp_sb[:], grp_ps[:])
                grp2 = small.tile([G, 2], F32, tag="grp2", name="grp2")  # (mean, rstd)
                var = small.tile([G, 1], F32, tag="var", name="var")
                epst = small.tile([G, 1], F32, tag="epst", name="epst")
                nc.vector.memset(epst[:], EPS)
                nc.vector.tensor_copy(grp2[:, 0:1], grp_sb[:, 0:1])
                nc.vector.tensor_tensor(var[:], grp_sb[:, 0:1], grp_sb[:, 0:1],
                                        mybir.AluOpType.mult)
                nc.vector.tensor_tensor(var[:], grp_sb[:, 1:2], var[:],
                                        mybir.AluOpType.subtract)
                nc.scalar.activation(var[:], var[:], mybir.ActivationFunctionType.Sqrt,
                                     bias=epst[:])
                nc.vector.reciprocal(grp2[:, 1:2], var[:])

                # broadcast to channels; A = rstd*gn_w, B = gn_b - mean*A
                for i in range(2):
                    mb_ps = psS.tile([128, 2], F32, tag="s", name="s")
                    nc.tensor.matmul(mb_ps[:], bsel[:, 128 * i:128 * (i + 1)], grp2[:],
                                     start=True, stop=True)
                    nc.vector.tensor_tensor(A[i][:], mb_ps[:, 1:2], gnw[i][:],
                                            mybir.AluOpType.mult)
                    nc.vector.tensor_tensor(Bs[i][:], mb_ps[:, 0:1], A[i][:],
                                            mybir.AluOpType.mult)
                    nc.vector.tensor_tensor(Bs[i][:], gnb[i][:], Bs[i][:],
                                            mybir.AluOpType.subtract)
                    # folded weights
                    nc.vector.tensor_scalar_mul(wtf[i][:], wt[i][:], A[i][:])

                # folded q/k biases: qkvb[o] + sum_c wT[c,o]*B[c]
                for o in range(4):
                    b_ps = psS.tile([128, 1], F32, tag="s", name="s")
                    nc.tensor.matmul(b_ps[:], wt[0][:, 128 * o:128 * (o + 1)], Bs[0][:],
                                     start=True, stop=False)
                    nc.tensor.matmul(b_ps[:], wt[1][:, 128 * o:128 * (o + 1)], Bs[1][:],
                                     start=False, stop=True)
                    nc.vector.tensor_tensor(qkb[o][:], qkb[o][:], b_ps[:],
                                            mybir.AluOpType.add)
                # folded v bias as a row [1, C], then broadcast to 128 partitions
                vb_ps = psS.tile([1, C], F32, tag="s", name="s")
                nc.tensor.matmul(vb_ps[:], Bs[0][:], wt[0][:, 2 * C:3 * C],
                                 start=True, stop=False)
                nc.tensor.matmul(vb_ps[:], Bs[1][:], wt[1][:, 2 * C:3 * C],
                                 start=False, stop=True)
                vb_row = small.tile([1, C], F32, tag="vb_row", name="vb_row")
                nc.vector.tensor_tensor(vb_row[:], vb_ps[:], vbrow[:],
                                        mybir.AluOpType.add)
                nc.gpsimd.partition_broadcast(vbias_bc[:], vb_row[:])

                # ================= q / k / vT projections ====================
                # q: rows 0..255 of qkv over the t-half
                for o in range(2):
                    for t in range(THALF // 512):
                        ts = slice(512 * t, 512 * (t + 1))
                        ps = psM.tile([128, 512], F32, tag="mm", name="mm")
                        nc.tensor.matmul(ps[:], _r(wtf[0][:, 128 * o:128 * (o + 1)]),
                                         _r(xq[0][:, ts]), start=True, stop=False)
                        nc.tensor.matmul(ps[:], _r(wtf[1][:, 128 * o:128 * (o + 1)]),
                                         _r(xq[1][:, ts]), start=False, stop=True)
                        nc.vector.tensor_scalar(q_sb[o][:, ts], ps[:], qkb[o][:], SCALE,
                                                mybir.AluOpType.add, mybir.AluOpType.mult)
                # k: rows 256..511 over full T
                for o in range(2):
                    for t in range(T // 512):
                        ts = slice(512 * t, 512 * (t + 1))
                        ps = psM.tile([128, 512], F32, tag="mm", name="mm")
                        nc.tensor.matmul(ps[:], _r(wtf[0][:, C + 128 * o:C + 128 * (o + 1)]),
                                         _r(x[0][:, ts]), start=True, stop=False)
                        nc.tensor.matmul(ps[:], _r(wtf[1][:, C + 128 * o:C + 128 * (o + 1)]),
                                         _r(x[1][:, ts]), start=False, stop=True)
                        nc.vector.tensor_scalar(k_sb[o][:, ts], ps[:], qkb[2 + o][:], SCALE,
                                                mybir.AluOpType.add, mybir.AluOpType.mult)
                # vT: [s, ch] = x[:, s-chunk].T @ wv  (s on partitions)
                nc.gpsimd.memset(vt[:, :, :, CH:CH + 1], 1.0)  # ones column
                for sc in range(T // 128):
                    ss = slice(128 * sc, 128 * (sc + 1))
                    ps = psM.tile([128, C], F32, tag="mm", name="mm")
                    nc.tensor.matmul(ps[:], _r(x[0][:, ss]), _r(wtf[0][:, 2 * C:3 * C]),
                                     start=True, stop=False)
                    nc.tensor.matmul(ps[:], _r(x[1][:, ss]), _r(wtf[1][:, 2 * C:3 * C]),
                                     start=False, stop=True)
                    nc.vector.tensor_tensor(
                        vt[:, sc, :, 0:CH],
                        ps[:].rearrange("p (h c) -> p h c", h=NH),
                        vbias_bc[:].rearrange("p (h c) -> p h c", h=NH),
                        mybir.AluOpType.add)

            # ================= attention + proj ==============================
            NSC = T // 128                      # 32 s-chunks
            groups = [(s, min(3, NSC - s)) for s in range(0, NSC, 3)]
            with tc.tile_pool(name="psQK", bufs=2, space="PSUM") as psQK, \
                 tc.tile_pool(name="psAV", bufs=2, space="PSUM") as psAV, \
                 tc.tile_pool(name="pexp", bufs=3) as pexp, \
                 tc.tile_pool(name="nrm", bufs=2) as nrm, \
                 tc.tile_pool(name="outp", bufs=3) as outp:
                for tb in range(THALF // 512):
                    tbs = slice(512 * tb, 512 * (tb + 1))
                    for h in range(NH):
                        qk_tile = q_sb[h // 2]
                        kk_tile = k_sb[h // 2]
                        hp = slice(64 * (h % 2), 64 * (h % 2) + 64)
                        acc = psAV.tile([128, 512], F32, tag="av", name="av")
                        for (s0, glen) in groups:
                            qkp = psQK.tile([128, 3, 512], F32, tag="qk", name="qk")
                            for j in range(glen):
                                sc = s0 + j
                                nc.tensor.matmul(
                                    qkp[:, j, :],
                                    _r(kk_tile[hp, 128 * sc:128 * (sc + 1)]),
                                    _r(qk_tile[hp, tbs]),
                                    start=True, stop=True)
                            pe = pexp.tile([128, 3, 512], P_DT, tag="pe", name="pe")
                            nc.scalar.activation(pe[:, 0:glen, :], qkp[:, 0:glen, :],
                                                 mybir.ActivationFunctionType.Exp)
                            for j in range(glen):
                                sc = s0 + j
                                nc.tensor.matmul(acc[0:CH + 1, :],
                                                 vt[:, sc, h, 0:CH + 1],
                                                 pe[:, j, :],
                                                 start=(sc == 0), stop=(sc == NSC - 1))
                        # normalize: a = acc[0:64] * (1/acc[64]) broadcast
                        rec = nrm.tile([1, 512], F32, tag="rec", name="rec")
                        nc.vector.reciprocal(rec[:], acc[CH:CH + 1, :])
                        bc = nrm.tile([CH, 512], F32, tag="bc", name="bc")
                        nc.gpsimd.partition_broadcast(bc[:], rec[:])
                        nc.vector.tensor_tensor(a_sb[h // 2][hp, tbs], acc[0:CH, :],
                                                bc[:], mybir.AluOpType.mult)
                    # proj + bias + residual for this t-block
                    for o in range(2):
                        pr = psAV.tile([128, 512], F32, tag="av", name="av")
                        nc.tensor.matmul(pr[:], _r(pjt[0][:, 128 * o:128 * (o + 1)]),
                                         _r(a_sb[0][:, tbs]), start=True, stop=False)
                        nc.tensor.matmul(pr[:], _r(pjt[1][:, 128 * o:128 * (o + 1)]),
                                         _r(a_sb[1][:, tbs]), start=False, stop=True)
                        res = outp.tile([128, 512], F32, tag="res", name="res")
                        nc.vector.tensor_scalar(res[:], pr[:], pjb[o][:], None,
                                                mybir.AluOpType.add)
                        nc.vector.tensor_tensor(res[:], res[:], xq[o][:, tbs].bitcast(F32),
                                                mybir.AluOpType.add)
                        nc.sync.dma_start(d_out[128 * o:128 * (o + 1), tbs], res[:])

    nc.compile()
    return nc


def _host_consts():
    g1 = np.zeros((128, G), dtype=np.float32)
    g2 = np.zeros((128, G), dtype=np.float32)
    for c in range(128):
        g1[c, c // CPG] = 1.0 / CPG
        g2[c, G // 2 + c // CPG] = 1.0 / CPG
    gsel = np.concatenate([g1, g2], axis=1)          # [128, 2G]
    bsel = np.zeros((G, C), dtype=np.float32)
    for c in range(C):
        bsel[c // CPG, c] = 1.0
    return gsel, bsel


def kernel(x, gn_w, gn_b, qkv_w, qkv_b, proj_w, proj_b):
    global LAST_RESULTS
    if "nc" not in _CACHE:
        _CACHE["nc"] = _build_program()
    nc = _CACHE["nc"]

    x = np.ascontiguousarray(np.asarray(x, dtype=np.float32))
    xr = x.reshape(B, C, T)
    gsel, bsel = _host_consts()
    shared = {
        "wT": np.ascontiguousarray(np.asarray(qkv_w, np.float32).T),
        "qkvb": np.asarray(qkv_b, np.float32).reshape(3 * C, 1).copy(),
        "vbrow": np.asarray(qkv_b, np.float32)[2 * C:].reshape(1, C).copy(),
        "pjT": np.ascontiguousarray(np.asarray(proj_w, np.float32).T),
        "pjb": np.asarray(proj_b, np.float32).reshape(C, 1).copy(),
        "gnw": np.asarray(gn_w, np.float32).reshape(C, 1).copy(),
        "gnb": np.asarray(gn_b, np.float32).reshape(C, 1).copy(),
        "gsel": gsel,
        "bsel": bsel,
    }
    in_maps = []
    for c in range(NCORES):
        b, hf = c // 2, c % 2
        m = dict(shared)
        m["xb"] = np.ascontiguousarray(xr[b])
        m["xq"] = np.ascontiguousarray(xr[b][:, hf * THALF:(hf + 1) * THALF])
        in_maps.append(m)

    res = bass_utils.run_bass_kernel_spmd(nc, in_maps, core_ids=list(range(NCORES)))
    LAST_RESULTS = res

    out = np.empty((B, C, T), dtype=np.float32)
    for c in range(NCORES):
        b, hf = c // 2, c % 2
        out[b][:, hf * THALF:(hf + 1) * THALF] = res.results[c]["out"]
    return out.reshape(B, C, HH, WW)


# revision 7
# speedup vs baseline: 1.1892x; 1.1892x over previous
"""AttentionBlock (GroupNorm + 1x1 QKV + MHA + proj + residual) on 8 trn2 cores.

Sharding: core c -> (batch b = c//2, t-half = c%2). Each core computes all 4
heads for its 2048 query positions; k/v are computed over the full T=4096 from
the core's batch. No cross-core communication needed.

Layout trick: attention scores are computed TRANSPOSED, S_T[s, t] (s on
partitions), so the AV matmul needs no transposes: a[ch, t] = vT[s, ch].T @
P_T[s, t]. The softmax denominator comes from a ones-column appended to vT.
GroupNorm is folded into the QKV weights (per-channel scale/shift).
"""

import math

import numpy as np

import concourse.bass as bass
import concourse.tile as tile
from concourse import bacc, mybir
from concourse import bass_utils

F32 = mybir.dt.float32
F32R = mybir.dt.float32r
BF16 = mybir.dt.bfloat16
F16 = mybir.dt.float16

B, C, HH, WW = 4, 256, 64, 64
T = HH * WW            # 4096
NH = 4                 # heads per batch
CH = C // NH           # 64 channels per head
G = 32                 # groupnorm groups
CPG = C // G           # 8 channels per group
EPS = 1e-5
NCORES = 8
THALF = T // 2         # 2048  t-columns per core
SCALE = 1.0 / math.sqrt(math.sqrt(CH))

USE_F32R = True        # f32r (tf32-like) for the big matmuls
P_DT = BF16            # dtype of exp'd attention weights + vT (bf16 or f32)

_CACHE = {}
LAST_RESULTS = None


def _r(ap):
    return ap


def _build_program():
    nc = bacc.Bacc("TRN2", target_bir_lowering=False, debug=False)

    d_xb = nc.dram_tensor("xb", [C, T], F32R, kind="ExternalInput").ap()
    d_xq = nc.dram_tensor("xq", [C, THALF], F32R, kind="ExternalInput").ap()
    d_wT = nc.dram_tensor("wT", [C, 3 * C], F32, kind="ExternalInput").ap()
    d_qkvb = nc.dram_tensor("qkvb", [3 * C, 1], F32, kind="ExternalInput").ap()
    d_vbrow = nc.dram_tensor("vbrow", [1, C], F32, kind="ExternalInput").ap()
    d_pjT = nc.dram_tensor("pjT", [C, C], F32R, kind="ExternalInput").ap()
    d_pjb = nc.dram_tensor("pjb", [C, 1], F32, kind="ExternalInput").ap()
    d_gnw = nc.dram_tensor("gnw", [C, 1], F32, kind="ExternalInput").ap()
    d_gnb = nc.dram_tensor("gnb", [C, 1], F32, kind="ExternalInput").ap()
    d_gsel = nc.dram_tensor("gsel", [128, 2 * G], F32, kind="ExternalInput").ap()
    d_bsel = nc.dram_tensor("bsel", [G, C], F32, kind="ExternalInput").ap()
    d_out = nc.dram_tensor("out", [C, THALF], F32, kind="ExternalOutput").ap()

    with tile.TileContext(nc) as tc:
        with tc.tile_pool(name="data", bufs=1) as data, \
             tc.tile_pool(name="small", bufs=1) as small, \
             tc.tile_pool(name="work", bufs=3) as work:

            # ---- persistent SBUF tensors ----
            x = [data.tile([128, T], F32R, tag=f"x{i}", name=f"x{i}") for i in range(2)]
            xq = [data.tile([128, THALF], F32R, tag=f"xq{i}", name=f"xq{i}") for i in range(2)]
            wt = [data.tile([128, 3 * C], F32, tag=f"wt{i}", name=f"wt{i}") for i in range(2)]
            wtf = [data.tile([128, 3 * C], F32R, tag=f"wtf{i}", name=f"wtf{i}") for i in range(2)]
            pjt = [data.tile([128, C], F32R, tag=f"pjt{i}", name=f"pjt{i}") for i in range(2)]
            q_sb = [data.tile([128, THALF], F16, tag=f"q{i}", name=f"q{i}") for i in range(2)]
            k_sb = [data.tile([128, T], F16, tag=f"k{i}", name=f"k{i}") for i in range(2)]
            # vT: per s-chunk, per head: 64 v-columns + 1 ones-column (+3 pad)
            vt = data.tile([128, T // 128, NH, CH + 4], P_DT, tag="vt", name="vt")
            a_sb = [data.tile([128, THALF], F32R, tag=f"a{i}", name=f"a{i}") for i in range(2)]
            vbias_bc = data.tile([128, C], F32, tag="vbias_bc", name="vbias_bc")

            gnw = [small.tile([128, 1], F32, tag=f"gnw{i}", name=f"gnw{i}") for i in range(2)]
            gnb = [small.tile([128, 1], F32, tag=f"gnb{i}", name=f"gnb{i}") for i in range(2)]
            pjb = [small.tile([128, 1], F32, tag=f"pjb{i}", name=f"pjb{i}") for i in range(2)]
            qkb = [small.tile([128, 1], F32, tag=f"qkb{o}", name=f"qkb{o}") for o in range(4)]
            gsel = small.tile([128, 2 * G], F32, tag="gsel", name="gsel")
            bsel = small.tile([G, C], F32, tag="bsel", name="bsel")
            vbrow = small.tile([1, C], F32, tag="vbrow", name="vbrow")

            for i in range(2):
                cs = slice(128 * i, 128 * (i + 1))
                nc.sync.dma_start(x[i][:], d_xb[cs, :])
                nc.sync.dma_start(xq[i][:], d_xq[cs, :])
                nc.sync.dma_start(wt[i][:], d_wT[cs, :])
                nc.sync.dma_start(pjt[i][:], d_pjT[cs, :])
                nc.sync.dma_start(gnw[i][:], d_gnw[cs, :])
                nc.sync.dma_start(gnb[i][:], d_gnb[cs, :])
                nc.sync.dma_start(pjb[i][:], d_pjb[cs, :])
            for o in range(4):
                nc.sync.dma_start(qkb[o][:], d_qkvb[128 * o:128 * (o + 1), :])
            nc.sync.dma_start(gsel[:], d_gsel[:, :])
            nc.sync.dma_start(bsel[:], d_bsel[:, :])
            nc.sync.dma_start(vbrow[:], d_vbrow[:, :])

            # ================= GroupNorm stats -> per-channel A, B ==========
            with tc.tile_pool(name="psS", bufs=1, space="PSUM") as psS, \
                 tc.tile_pool(name="psM", bufs=2, space="PSUM") as psM:

                A = [small.tile([128, 1], F32, tag=f"A{i}", name=f"A{i}") for i in range(2)]
                Bs = [small.tile([128, 1], F32, tag=f"B{i}", name=f"B{i}") for i in range(2)]
                pcs = [small.tile([128, 2], F32, tag=f"pcs{i}", name=f"pcs{i}") for i in range(2)]
                for i in range(2):
                    stats = work.tile([128, 8, 6], F32, tag="bnstats", name="bnstats")
                    for j in range(8):
                        nc.vector.bn_stats(stats[:, j, :], x[i][:, 512 * j:512 * (j + 1)].bitcast(F32))
                    mv = work.tile([128, 2], F32, tag="bnmv", name="bnmv")
                    nc.vector.bn_aggr(mv[:], stats[:])
                    # pcs = (mean, E[x^2]) per channel
                    nc.vector.tensor_copy(pcs[i][:, 0:1], mv[:, 0:1])
                    nc.vector.tensor_tensor(pcs[i][:, 1:2], mv[:, 0:1], mv[:, 0:1],
                                            mybir.AluOpType.mult)
                    nc.vector.tensor_tensor(pcs[i][:, 1:2], pcs[i][:, 1:2], mv[:, 1:2],
                                            mybir.AluOpType.add)

                # group stats [G, 2] = (mean_g, E[x^2]_g)
                grp_ps = psS.tile([G, 2], F32, tag="s", name="s")
                nc.tensor.matmul(grp_ps[:], gsel[:, 0:G], pcs[0][:], start=True, stop=False)
                nc.tensor.matmul(grp_ps[:], gsel[:, G:2 * G], pcs[1][:], start=False, stop=True)

                grp_sb = small.tile([G, 2], F32, tag="grp_sb", name="grp_sb")
                nc.vector.tensor_copy(grp_sb[:], grp_ps[:])
                grp2 = small.tile([G, 2], F32, tag="grp2", name="grp2")  # (mean, rstd)
                var = small.tile([G, 1], F32, tag="var", name="var")
                epst = small.tile([G, 1], F32, tag="epst", name="epst")
                nc.vector.memset(epst[:], EPS)
                nc.vector.tensor_copy(grp2[:, 0:1], grp_sb[:, 0:1])
                nc.vector.tensor_tensor(var[:], grp_sb[:, 0:1], grp_sb[:, 0:1],
                                        mybir.AluOpType.mult)
                nc.vector.tensor_tensor(var[:], grp_sb[:, 1:2], var[:],
                                        mybir.AluOpType.subtract)
                nc.scalar.activation(var[:], var[:], mybir.ActivationFunctionType.Sqrt,
                                     bias=epst[:])
                nc.vector.reciprocal(grp2[:, 1:2], var[:])

                # broadcast to channels; A = rstd*gn_w, B = gn_b - mean*A
                for i in range(2):
                    mb_ps = psS.tile([128, 2], F32, tag="s", name="s")
                    nc.tensor.matmul(mb_ps[:], bsel[:, 128 * i:128 * (i + 1)], grp2[:],
                                     start=True, stop=True)
                    nc.vector.tensor_tensor(A[i][:], mb_ps[:, 1:2], gnw[i][:],
                                            mybir.AluOpType.mult)
                    nc.vector.tensor_tensor(Bs[i][:], mb_ps[:, 0:1], A[i][:],
                                            mybir.AluOpType.mult)
                    nc.vector.tensor_tensor(Bs[i][:], gnb[i][:], Bs[i][:],
                                            mybir.AluOpType.subtract)
                    # folded weights
                    nc.vector.tensor_scalar_mul(wtf[i][:], wt[i][:], A[i][:])

                # folded q/k biases: qkvb[o] + sum_c wT[c,o]*B[c]
                for o in range(4):
                    b_ps = psS.tile([128, 1], F32, tag="s", name="s")
                    nc.tensor.matmul(b_ps[:], wt[0][:, 128 * o:128 * (o + 1)], Bs[0][:],
                                     start=True, stop=False)
                    nc.tensor.matmul(b_ps[:], wt[1][:, 128 * o:128 * (o + 1)], Bs[1][:],
                                     start=False, stop=True)
                    nc.vector.tensor_tensor(qkb[o][:], qkb[o][:], b_ps[:],
                                            mybir.AluOpType.add)
                # folded v bias as a row [1, C], then broadcast to 128 partitions
                vb_ps = psS.tile([1, C], F32, tag="s", name="s")
                nc.tensor.matmul(vb_ps[:], Bs[0][:], wt[0][:, 2 * C:3 * C],
                                 start=True, stop=False)
                nc.tensor.matmul(vb_ps[:], Bs[1][:], wt[1][:, 2 * C:3 * C],
                                 start=False, stop=True)
                vb_row = small.tile([1, C], F32, tag="vb_row", name="vb_row")
                nc.vector.tensor_tensor(vb_row[:], vb_ps[:], vbrow[:],
                                        mybir.AluOpType.add)
                nc.gpsimd.partition_broadcast(vbias_bc[:], vb_row[:])

                # ================= q / k / vT projections ====================
                # q: rows 0..255 of qkv over the t-half
                for o in range(2):
                    for t in range(THALF // 512):
                        ts = slice(512 * t, 512 * (t + 1))
                        ps = psM.tile([128, 512], F32, tag="mm", name="mm")
                        nc.tensor.matmul(ps[:], _r(wtf[0][:, 128 * o:128 * (o + 1)]),
                                         _r(xq[0][:, ts]), start=True, stop=False)
                        nc.tensor.matmul(ps[:], _r(wtf[1][:, 128 * o:128 * (o + 1)]),
                                         _r(xq[1][:, ts]), start=False, stop=True)
                        nc.vector.tensor_scalar(q_sb[o][:, ts], ps[:], qkb[o][:], SCALE,
                                                mybir.AluOpType.add, mybir.AluOpType.mult)
                # k: rows 256..511 over full T
                for o in range(2):
                    for t in range(T // 512):
                        ts = slice(512 * t, 512 * (t + 1))
                        ps = psM.tile([128, 512], F32, tag="mm", name="mm")
                        nc.tensor.matmul(ps[:], _r(wtf[0][:, C + 128 * o:C + 128 * (o + 1)]),
                                         _r(x[0][:, ts]), start=True, stop=False)
                        nc.tensor.matmul(ps[:], _r(wtf[1][:, C + 128 * o:C + 128 * (o + 1)]),
                                         _r(x[1][:, ts]), start=False, stop=True)
                        nc.vector.tensor_scalar(k_sb[o][:, ts], ps[:], qkb[2 + o][:], SCALE,
                                                mybir.AluOpType.add, mybir.AluOpType.mult)
                # vT: [s, ch] = x[:, s-chunk].T @ wv  (s on partitions)
                nc.gpsimd.memset(vt[:, :, :, CH:CH + 1], 1.0)  # ones column
                for sc in range(T // 128):
                    ss = slice(128 * sc, 128 * (sc + 1))
                    ps = psM.tile([128, C], F32, tag="mm", name="mm")
                    nc.tensor.matmul(ps[:], _r(x[0][:, ss]), _r(wtf[0][:, 2 * C:3 * C]),
                                     start=True, stop=False)
                    nc.tensor.matmul(ps[:], _r(x[1][:, ss]), _r(wtf[1][:, 2 * C:3 * C]),
                                     start=False, stop=True)
                    nc.vector.tensor_tensor(
                        vt[:, sc, :, 0:CH],
                        ps[:].rearrange("p (h c) -> p h c", h=NH),
                        vbias_bc[:].rearrange("p (h c) -> p h c", h=NH),
                        mybir.AluOpType.add)

            # ================= attention + proj ==============================
            NSC = T // 128                      # 32 s-chunks
            groups = [(s, min(3, NSC - s)) for s in range(0, NSC, 3)]
            with tc.tile_pool(name="psQK", bufs=2, space="PSUM") as psQK, \
                 tc.tile_pool(name="psAV", bufs=2, space="PSUM") as psAV, \
                 tc.tile_pool(name="pexp", bufs=3) as pexp, \
                 tc.tile_pool(name="nrm", bufs=2) as nrm, \
                 tc.tile_pool(name="outp", bufs=3) as outp:
                for tb in range(THALF // 512):
                    tbs = slice(512 * tb, 512 * (tb + 1))
                    for h in range(NH):
                        qk_tile = q_sb[h // 2]
                        kk_tile = k_sb[h // 2]
                        hp = slice(64 * (h % 2), 64 * (h % 2) + 64)
                        acc = psAV.tile([128, 512], F32, tag="av", name="av")
                        for (s0, glen) in groups:
                            qkp = psQK.tile([128, 3, 512], F32, tag="qk", name="qk")
                            for j in range(glen):
                                sc = s0 + j
                                nc.tensor.matmul(
                                    qkp[:, j, :],
                                    _r(kk_tile[hp, 128 * sc:128 * (sc + 1)]),
                                    _r(qk_tile[hp, tbs]),
                                    start=True, stop=True)
                            pe = pexp.tile([128, 3, 512], P_DT, tag="pe", name="pe")
                            nc.scalar.activation(pe[:, 0:glen, :], qkp[:, 0:glen, :],
                                                 mybir.ActivationFunctionType.Exp)
                            for j in range(glen):
                                sc = s0 + j
                                nc.tensor.matmul(acc[0:CH + 1, :],
                                                 vt[:, sc, h, 0:CH + 1],
                                                 pe[:, j, :],
                                                 start=(sc == 0), stop=(sc == NSC - 1))
                        # normalize: a = acc[0:64] * (1/acc[64]) broadcast
                        rec = nrm.tile([1, 512], F32, tag="rec", name="rec")
                        nc.vector.reciprocal(rec[:], acc[CH:CH + 1, :])
                        bc = nrm.tile([CH, 512], F32, tag="bc", name="bc")
                        nc.gpsimd.partition_broadcast(bc[:], rec[:])
                        nc.vector.tensor_tensor(a_sb[h // 2][hp, tbs], acc[0:CH, :],
                                                bc[:], mybir.AluOpType.mult)
                    # proj + bias + residual for this t-block
                    for o in range(2):
                        pr = psAV.tile([128, 512], F32, tag="av", name="av")
                        nc.tensor.matmul(pr[:], _r(pjt[0][:, 128 * o:128 * (o + 1)]),
                                         _r(a_sb[0][:, tbs]), start=True, stop=False)
                        nc.tensor.matmul(pr[:], _r(pjt[1][:, 128 * o:128 * (o + 1)]),
                                         _r(a_sb[1][:, tbs]), start=False, stop=True)
                        res = outp.tile([128, 512], F32, tag="res", name="res")
                        nc.vector.tensor_scalar(res[:], pr[:], pjb[o][:], None,
                                                mybir.AluOpType.add)
                        nc.vector.tensor_tensor(res[:], res[:], xq[o][:, tbs].bitcast(F32),
                                                mybir.AluOpType.add)
                        nc.sync.dma_start(d_out[128 * o:128 * (o + 1), tbs], res[:])

    nc.compile()
    return nc


def _host_consts():
    g1 = np.zeros((128, G), dtype=np.float32)
    g2 = np.zeros((128, G), dtype=np.float32)
    for c in range(128):
        g1[c, c // CPG] = 1.0 / CPG
        g2[c, G // 2 + c // CPG] = 1.0 / CPG
    gsel = np.concatenate([g1, g2], axis=1)          # [128, 2G]
    bsel = np.zeros((G, C), dtype=np.float32)
    for c in range(C):
        bsel[c // CPG, c] = 1.0
    return gsel, bsel


def kernel(x, gn_w, gn_b, qkv_w, qkv_b, proj_w, proj_b):
    global LAST_RESULTS
    if "nc" not in _CACHE:
        _CACHE["nc"] = _build_program()
    nc = _CACHE["nc"]

    x = np.ascontiguousarray(np.asarray(x, dtype=np.float32))
    xr = x.reshape(B, C, T)
    gsel, bsel = _host_consts()
    shared = {
        "wT": np.ascontiguousarray(np.asarray(qkv_w, np.float32).T),
        "qkvb": np.asarray(qkv_b, np.float32).reshape(3 * C, 1).copy(),
        "vbrow": np.asarray(qkv_b, np.float32)[2 * C:].reshape(1, C).copy(),
        "pjT": np.ascontiguousarray(np.asarray(proj_w, np.float32).T),
        "pjb": np.asarray(proj_b, np.float32).reshape(C, 1).copy(),
        "gnw": np.asarray(gn_w, np.float32).reshape(C, 1).copy(),
        "gnb": np.asarray(gn_b, np.float32).reshape(C, 1).copy(),
        "gsel": gsel,
        "bsel": bsel,
    }
    in_maps = []
    for c in range(NCORES):
        b, hf = c // 2, c % 2
        m = dict(shared)
        m["xb"] = np.ascontiguousarray(xr[b])
        m["xq"] = np.ascontiguousarray(xr[b][:, hf * THALF:(hf + 1) * THALF])
        in_maps.append(m)

    res = bass_utils.run_bass_kernel_spmd(nc, in_maps, core_ids=list(range(NCORES)))
    LAST_RESULTS = res

    out = np.empty((B, C, T), dtype=np.float32)
    for c in range(NCORES):
        b, hf = c // 2, c % 2
        out[b][:, hf * THALF:(hf + 1) * THALF] = res.results[c]["out"]
    return out.reshape(B, C, HH, WW)
